# revision 6
# baseline (speedup 1.0000x reference)
"""KANLinear Trainium2 kernel, two-stage variant.

Stage 1 (fp32, on PE): per group of 9 input features, compute the 11 cubic
B-spline basis values from 14 truncated-power features via a banded
4th-difference matrix Jb: B[(il,j), b] = sum_q Jb[(il,q),(il,j)] * r[(il,q), b],
with r = relu(u-q)^3 computed elementwise in the (il,q)-packed partition
layout.  The catastrophic cancellation happens inside fp32 PSUM, so the
resulting basis values are small and well-conditioned.

Stage 2 (fp16, on PE): one dense matmul of the basis against coef*scale_sp
plus the silu residual path.  No hi/lo splitting needed.

Sharding: data-parallel over batch, 512 rows per core.
"""
import numpy as np
from contextlib import ExitStack

NCORES = 8
B_CORE = 512
IN = 512
OUT = 512
NQ = 14          # truncated-power features per input
NJ = 11          # basis functions per input
GI = 9           # inputs per stage-1 group
NG = 57          # ceil(512/9); last group has 8 inputs
SC = None


def _build_program(t0, h):
    from concourse import bacc, tile, mybir
    dt = mybir.dt
    AF = mybir.ActivationFunctionType
    OP = mybir.AluOpType

    nc = bacc.Bacc()
    xr_p = nc.declare_dram_parameter("xr", [NG, GI * NQ, B_CORE], dt.float32, isOutput=False)
    jb_p = nc.declare_dram_parameter("Jb", [GI * NQ, GI * NJ], dt.float32, isOutput=False)
    qb_p = nc.declare_dram_parameter("qb", [GI * NQ, 1], dt.float32, isOutput=False)
    w2_p = nc.declare_dram_parameter("W2", [NG, GI * NJ, OUT], dt.float16, isOutput=False)
    xT_p = nc.declare_dram_parameter("xT", [IN, B_CORE], dt.float32, isOutput=False)
    ws_p = nc.declare_dram_parameter("Ws", [4, 128, OUT], dt.float16, isOutput=False)
    y_p = nc.declare_dram_parameter("y", [OUT, B_CORE], dt.float32, isOutput=True)

    f32, f16 = dt.float32, dt.float16
    P1 = GI * NQ   # 126
    M1 = GI * NJ   # 99
    with ExitStack() as ctx:
        tc = ctx.enter_context(tile.TileContext(nc))
        sb = ctx.enter_context(tc.tile_pool(name="sb", bufs=2))
        wp = ctx.enter_context(tc.tile_pool(name="wp", bufs=4))
        fp = ctx.enter_context(tc.tile_pool(name="fp", bufs=3))
        ps = ctx.enter_context(tc.tile_pool(name="ps", bufs=1, space="PSUM"))
        p1 = ctx.enter_context(tc.tile_pool(name="p1", bufs=3, space="PSUM"))

        jb_sb = sb.tile([P1, M1], f32, tag="jb", bufs=1)
        nc.sync.dma_start(jb_sb[:], jb_p[:])
        qb_sb = sb.tile([P1, 1], f32, tag="qb", bufs=1)
        nc.sync.dma_start(qb_sb[:], qb_p[:])

        ps_y = [ps.tile([128, B_CORE], f32, tag=f"y{o}", name=f"ps_y{o}") for o in range(4)]
        first = [True] * 4

        for g in range(NG):
            pp = P1 if g < NG - 1 else 8 * NQ
            mm = M1 if g < NG - 1 else 8 * NJ
            xr = fp.tile([P1, B_CORE], f32, tag="xr")
            nc.sync.dma_start(xr[:pp], xr_p[g, :pp])
            # relu((x-t0)/h - q) via the ACT free affine; no clamp needed:
            # the 4th difference annihilates cubics, so u outside [0,14]
            # yields ~0 basis values automatically.
            rl = fp.tile([P1, B_CORE], f32, tag="rl")
            nc.scalar.activation(rl[:pp], xr[:pp], AF.Relu, bias=qb_sb[:pp], scale=1.0 / h)
            sq = fp.tile([P1, B_CORE], f32, tag="sq")
            nc.scalar.activation(sq[:pp], xr[:pp], AF.Square, bias=qb_sb[:pp], scale=1.0 / h)
            rr = fp.tile([P1, B_CORE], f32, tag="rr")
            nc.vector.tensor_tensor(rr[:pp], rl[:pp], sq[:pp], OP.mult)
            bps = p1.tile([M1, B_CORE], f32, tag="bps")
            nc.tensor.matmul(bps[:mm], lhsT=jb_sb[:pp, :mm], rhs=rr[:pp],
                             start=True, stop=True)
            bt = fp.tile([M1, B_CORE], f16, tag="bt")
            nc.vector.tensor_copy(bt[:mm], bps[:mm])
            w2 = wp.tile([M1, OUT], f16, tag="w2")
            nc.sync.dma_start(w2[:mm], w2_p[g, :mm])
            for oc in range(4):
                nc.tensor.matmul(ps_y[oc][:], lhsT=w2[:mm, oc * 128:(oc + 1) * 128],
                                 rhs=bt[:mm], start=first[oc], stop=False)
                first[oc] = False

        # silu residual path: x in (p, (g,b)) layout
        x_sb = sb.tile([128, 4 * B_CORE], f32, tag="x")
        nc.sync.dma_start(x_sb[:].rearrange("p (g b) -> p g b", g=4),
                          xT_p[:].rearrange("(g p) b -> p g b", p=128))
        s_sb = sb.tile([128, 4 * B_CORE], f16, tag="s")
        nc.scalar.activation(s_sb[:], x_sb[:], AF.Silu)
        for ig in range(4):
            ws = wp.tile([128, OUT], f16, tag="ws")
            nc.sync.dma_start(ws[:], ws_p[ig])
            s_s = s_sb[:, ig * B_CORE:(ig + 1) * B_CORE]
            for oc in range(4):
                nc.tensor.matmul(ps_y[oc][:], lhsT=ws[:, oc * 128:(oc + 1) * 128],
                                 rhs=s_s, start=False, stop=(ig == 3))

        for oc in range(4):
            y_t = sb.tile([128, B_CORE], f32, tag="y_t")
            nc.vector.tensor_copy(y_t[:], ps_y[oc][:])
            nc.sync.dma_start(y_p[oc * 128:(oc + 1) * 128, :], y_t[:])

    nc.compile()
    return nc


def kernel(x, grid, coef, scale_base, scale_sp, k=3, **_):
    from concourse.bass_utils import run_bass_kernel_spmd

    x = np.asarray(x, np.float32)
    grid = np.asarray(grid, np.float32)
    coef = np.asarray(coef)
    scale_base = np.asarray(scale_base)
    scale_sp = np.asarray(scale_sp)

    t0 = float(grid[0, 0])
    h = float(grid[0, 1] - grid[0, 0])

    # banded 4th-difference matrix (shared across groups), 1/6 folded in
    J = (1.0, -4.0, 6.0, -4.0, 1.0)
    Jb = np.zeros((GI * NQ, GI * NJ), np.float64)
    for il in range(GI):
        for j in range(NJ):
            for d in range(5):
                q = j + d
                if q < NQ:  # r_14 == 0 under the clamp
                    Jb[il * NQ + q, il * NJ + j] = J[d] / 6.0
    Jb = Jb.astype(np.float32)
    # bias per partition: -(t0/h) - q
    qb = (-t0 / h - np.tile(np.arange(NQ, dtype=np.float64), GI))[:, None].astype(np.float32)

    # stage-2 weights: W2[(g,il,j), o] = coef[i,o,j]*scale_sp[i,o], i = 9g+il
    ct = (coef.astype(np.float64) * scale_sp.astype(np.float64)[:, :, None])
    W2 = np.zeros((NG, GI * NJ, OUT), np.float64)
    for g in range(NG):
        ni = min(GI, IN - g * GI)
        blk = ct[g * GI:g * GI + ni].transpose(0, 2, 1)       # (ni, NJ, OUT)
        W2[g, :ni * NJ] = blk.reshape(ni * NJ, OUT)
    W2 = W2.astype(np.float16)
    Ws = np.ascontiguousarray(scale_base.astype(np.float16).reshape(4, 128, OUT))

    key = (t0, h)
    if getattr(kernel, "_nc_key", None) == key:
        nc = kernel._nc
    else:
        nc = _build_program(t0, h)
        kernel._nc = nc
        kernel._nc_key = key

    # replicated x rows: xr[g, il*NQ+q, b] = x[b, 9g+il]  (same for all q)
    in_maps = []
    for c in range(NCORES):
        xc = x[c * B_CORE:(c + 1) * B_CORE]           # (512 b, 512 i)
        xcT = np.ascontiguousarray(xc.T)               # (512 i, 512 b)
        # clamp to the knot span so r_14 == 0 exactly (tap dropped from Jb)
        xclip = np.clip(xcT, t0, t0 + NQ * h).astype(np.float32)
        xr = np.zeros((NG, GI * NQ, B_CORE), np.float32)
        for g in range(NG):
            ni = min(GI, IN - g * GI)
            xr[g, :ni * NQ] = np.repeat(xclip[g * GI:g * GI + ni], NQ, axis=0)
        in_maps.append({"xr": xr, "Jb": Jb, "qb": qb, "W2": W2,
                        "xT": xcT, "Ws": Ws})
    r = run_bass_kernel_spmd(nc, in_maps, list(range(NCORES)))
    kernel._last = r
    res = r.results
    y = np.concatenate([np.asarray(res[c]["y"]).T for c in range(NCORES)], axis=0)
    return np.ascontiguousarray(y.astype(np.float32))
